# revision 1
# baseline (speedup 1.0000x reference)
"""DNN MVDR Beamformer — Trainium2, 8 NeuronCores.

Strategy: data-parallel over B (B=8 -> one batch element per core) via
jax shard_map on the 8 axon-attached NeuronCores. The tiny MLP params are
replicated. The per-core computation (PSD estimation, attention reference,
MVDR solve, beamforming) is expressed in real-arithmetic JAX (no complex
dtypes, no LAPACK custom calls) so it lowers cleanly through neuronx-cc.
The 8x8 Hermitian solve inv(psd_n) @ psd_s is done with an explicit
Gauss-Jordan elimination on the augmented system, vectorized over (F,).
"""

import numpy as np

EPS = 1e-15
SCALING = 2.0
B, T, C, F, A = 8, 512, 8, 257, 320

_JAX_FN = None  # compiled shard_map callable, built lazily


def _build_jax_fn():
    import jax
    import jax.numpy as jnp
    from jax.sharding import Mesh, PartitionSpec as P
    from jax.experimental.shard_map import shard_map

    def per_core(data_real, data_imag, mask_speech, mask_noise,
                 mlp_w, mlp_b, gvec_w, gvec_b):
        # shapes (per core): data (1,T,C,F), masks (1,F,C,T)
        dr = jnp.transpose(data_real[0], (2, 1, 0))   # (F, C, T)
        di = jnp.transpose(data_imag[0], (2, 1, 0))   # (F, C, T)

        def psd(mask):
            m = mask[0].mean(axis=-2)                              # (F, T)
            m = m / (m.sum(axis=-1, keepdims=True) + EPS)          # (F, T)
            wr = dr * m[:, None, :]                                # (F, C, T)
            wi = di * m[:, None, :]
            # S[f,c,e] = sum_t w[f,c,t] * conj(x)[f,e,t]
            sr = jnp.einsum('fct,fet->fce', wr, dr) + jnp.einsum('fct,fet->fce', wi, di)
            si = jnp.einsum('fct,fet->fce', wi, dr) - jnp.einsum('fct,fet->fce', wr, di)
            return sr, si

        psd_s_r, psd_s_i = psd(mask_speech)
        psd_n_r, psd_n_i = psd(mask_noise)

        # ---- attention reference -> u (C,) ----
        eye = jnp.eye(C, dtype=jnp.float32)
        zr = psd_s_r * (1.0 - eye)                                 # zero diagonal
        zi = psd_s_i * (1.0 - eye)
        pr = zr.sum(axis=-1) / (C - 1)                             # (F, C)
        pi = zi.sum(axis=-1) / (C - 1)
        feat = jnp.sqrt(pr * pr + pi * pi)                         # (F, C)
        # psd_feat is (C, F) in reference (swapaxes); feat.T @ mlp_w
        mlp = jnp.tanh(feat.T @ mlp_w + mlp_b)                     # (C, A)
        e = (mlp @ gvec_w)[:, 0] + gvec_b[0]                       # (C,)
        e = SCALING * e
        e = e - e.max()
        ex = jnp.exp(e)
        u = ex / ex.sum()                                          # (C,)

        # ---- MVDR: solve psd_n @ X = psd_s per f (8x8 complex) ----
        # Augmented Gauss-Jordan, vectorized over F. Complex as (r, i).
        ar, ai = psd_n_r, psd_n_i                                  # (F, C, C)
        xr, xi = psd_s_r, psd_s_i                                  # (F, C, C)

        def gj_step(k, carry):
            ar, ai, xr, xi = carry
            prr = ar[:, k, :]                                      # (F, C) pivot row
            pri = ai[:, k, :]
            pxr = xr[:, k, :]
            pxi = xi[:, k, :]
            d = prr[:, k] ** 2 + pri[:, k] ** 2                    # (F,)
            inv_r = prr[:, k] / d
            inv_i = -pri[:, k] / d
            # scaled pivot row (complex multiply by inv)
            srr = prr * inv_r[:, None] - pri * inv_i[:, None]
            sri = prr * inv_i[:, None] + pri * inv_r[:, None]
            sxr = pxr * inv_r[:, None] - pxi * inv_i[:, None]
            sxi = pxr * inv_i[:, None] + pxi * inv_r[:, None]
            # column factors (zeroed at row k)
            fr = ar[:, :, k] * (1.0 - eye[k])[None, :]             # (F, C)
            fi = ai[:, :, k] * (1.0 - eye[k])[None, :]
            # rank-1 elimination, then write scaled pivot row back
            new_ar = ar - (fr[:, :, None] * srr[:, None, :] - fi[:, :, None] * sri[:, None, :])
            new_ai = ai - (fr[:, :, None] * sri[:, None, :] + fi[:, :, None] * srr[:, None, :])
            new_xr = xr - (fr[:, :, None] * sxr[:, None, :] - fi[:, :, None] * sxi[:, None, :])
            new_xi = xi - (fr[:, :, None] * sxi[:, None, :] + fi[:, :, None] * sxr[:, None, :])
            new_ar = new_ar.at[:, k, :].set(srr)
            new_ai = new_ai.at[:, k, :].set(sri)
            new_xr = new_xr.at[:, k, :].set(sxr)
            new_xi = new_xi.at[:, k, :].set(sxi)
            return new_ar, new_ai, new_xr, new_xi

        for k in range(C):
            ar, ai, xr, xi = gj_step(k, (ar, ai, xr, xi))
        num_r, num_i = xr, xi                                      # (F, C, C) = inv(N) @ S

        tr_r = jnp.trace(num_r, axis1=-2, axis2=-1)                # (F,)
        tr_i = jnp.trace(num_i, axis1=-2, axis2=-1)
        den = tr_r ** 2 + tr_i ** 2 + EPS
        itr_r = (tr_r + EPS) / den
        itr_i = -tr_i / den
        wsm_r = num_r * itr_r[:, None, None] - num_i * itr_i[:, None, None]
        wsm_i = num_r * itr_i[:, None, None] + num_i * itr_r[:, None, None]
        # ws[f,e] = sum_c ws_mat[f,e,c] * u[c]   (u real)
        ws_r = wsm_r @ u                                           # (F, C)
        ws_i = wsm_i @ u

        # ---- beamform: enh[f,t] = sum_c conj(ws)[f,c] x[f,c,t] ----
        enh_r = jnp.einsum('fc,fct->ft', ws_r, dr) + jnp.einsum('fc,fct->ft', ws_i, di)
        enh_i = jnp.einsum('fc,fct->ft', ws_r, di) - jnp.einsum('fc,fct->ft', ws_i, dr)
        out = jnp.stack([enh_r.T, enh_i.T], axis=-1)               # (T, F, 2)
        return out[None]                                           # (1, T, F, 2)

    devices = jax.devices()[:8]
    mesh = Mesh(np.asarray(devices), ("b",))
    fn = jax.jit(shard_map(
        per_core, mesh=mesh,
        in_specs=(P("b"), P("b"), P("b"), P("b"), P(), P(), P(), P()),
        out_specs=P("b"),
        check_rep=False,
    ))
    return fn


def _kernel_host(data_real, data_imag, mask_speech, mask_noise,
                 mlp_w, mlp_b, gvec_w, gvec_b):
    """Numpy fallback (same math, float64-free)."""
    data = np.transpose(data_real + 1j * data_imag, (0, 3, 2, 1)).astype(np.complex64)

    def psd(mask):
        m = mask.mean(axis=-2)
        m = m / (m.sum(axis=-1, keepdims=True) + EPS)
        return np.einsum('bfct,bft,bfet->bfce', data, m.astype(data.dtype),
                         np.conj(data))

    psd_s = psd(mask_speech)
    psd_n = psd(mask_noise)

    eye = np.eye(C, dtype=bool)
    z = np.where(eye, np.zeros((), psd_s.dtype), psd_s)
    p = np.swapaxes(z.sum(axis=-1) / (C - 1), -1, -2)
    feat = np.sqrt(p.real ** 2 + p.imag ** 2)
    mlp = np.tanh(feat @ mlp_w + mlp_b)
    e = (mlp @ gvec_w)[..., 0] + gvec_b[0]
    e = SCALING * e
    e = e - e.max(axis=-1, keepdims=True)
    ex = np.exp(e)
    u = ex / ex.sum(axis=-1, keepdims=True)

    num = np.linalg.inv(psd_n.astype(np.complex128)).astype(np.complex64) @ psd_s
    tr = np.einsum('bfcc->bf', num)
    wsm = num / (tr[..., None, None] + EPS)
    ws = np.einsum('bfec,bc->bfe', wsm, u.astype(wsm.dtype))
    enh = np.einsum('bfc,bfct->bft', np.conj(ws), data)
    enh = np.swapaxes(enh, -1, -2)
    return np.stack([enh.real, enh.imag], axis=-1).astype(np.float32)


def kernel(data_real, data_imag, mask_speech, mask_noise,
           mlp_w, mlp_b, gvec_w, gvec_b, ilens=None, **_unused):
    global _JAX_FN
    args = [np.asarray(a, np.float32) for a in
            (data_real, data_imag, mask_speech, mask_noise,
             mlp_w, mlp_b, gvec_w, gvec_b)]
    try:
        if _JAX_FN is None:
            _JAX_FN = _build_jax_fn()
        out = np.asarray(_JAX_FN(*args))
        return out.astype(np.float32)
    except Exception:
        return _kernel_host(*args)


# revision 2
# speedup vs baseline: 1.5129x; 1.5129x over previous
"""DNN MVDR Beamformer — Trainium2, 8 NeuronCores.

Strategy: data-parallel over B (B=8 -> one batch element per core) via
jax shard_map on the 8 axon-attached NeuronCores; tiny MLP params
replicated. Per-core computation (PSD estimation, attention reference,
MVDR Gauss-Jordan solve, beamforming) is expressed in real-arithmetic
JAX (no complex dtypes, no LAPACK custom calls) so it lowers cleanly
through neuronx-cc. Contractions batched over the independent F axis are
written as broadcast-multiply + reduce (vector-engine friendly) instead
of F-batched 8x8 matmuls, which are a pathological shape for the PE.

The normalized mask mean over channels (a 1.3%-of-FLOPs reduction) is
taken on host before shipping, halving host->device transfer volume.
"""

import numpy as np

EPS = 1e-15
SCALING = 2.0
B, T, C, F, A = 8, 512, 8, 257, 320

_JAX_FN = None  # compiled shard_map callable, built lazily


def _build_jax_fn():
    import jax
    import jax.numpy as jnp
    from jax.sharding import Mesh, PartitionSpec as P
    from jax.experimental.shard_map import shard_map

    def per_core(data_real, data_imag, m_speech, m_noise,
                 mlp_w, mlp_b, gvec_w, gvec_b):
        # per-core shapes: data (1,T,C,F), masks (1,F,T) pre-normalized
        dr = jnp.transpose(data_real[0], (2, 1, 0))   # (F, C, T)
        di = jnp.transpose(data_imag[0], (2, 1, 0))   # (F, C, T)

        def psd(m):
            # m: (F, T) normalized weights
            wr = dr * m[:, None, :]                                # (F, C, T)
            wi = di * m[:, None, :]
            # S[f,c,e] = sum_t w[f,c,t] * x[f,e,t]  (broadcast-mul + reduce)
            sr = (wr[:, :, None, :] * dr[:, None, :, :]).sum(-1) \
               + (wi[:, :, None, :] * di[:, None, :, :]).sum(-1)
            si = (wi[:, :, None, :] * dr[:, None, :, :]).sum(-1) \
               - (wr[:, :, None, :] * di[:, None, :, :]).sum(-1)
            return sr, si

        psd_s_r, psd_s_i = psd(m_speech[0])
        psd_n_r, psd_n_i = psd(m_noise[0])

        # ---- attention reference -> u (C,) ----
        eye = jnp.eye(C, dtype=jnp.float32)
        zr = psd_s_r * (1.0 - eye)                                 # zero diagonal
        zi = psd_s_i * (1.0 - eye)
        pr = zr.sum(axis=-1) / (C - 1)                             # (F, C)
        pi = zi.sum(axis=-1) / (C - 1)
        feat = jnp.sqrt(pr * pr + pi * pi)                         # (F, C)
        mlp = jnp.tanh(feat.T @ mlp_w + mlp_b)                     # (C, A)
        e = (mlp @ gvec_w)[:, 0] + gvec_b[0]                       # (C,)
        e = SCALING * e
        e = e - e.max()
        ex = jnp.exp(e)
        u = ex / ex.sum()                                          # (C,)

        # ---- MVDR: solve psd_n @ X = psd_s per f (8x8 complex GJ) ----
        ar, ai = psd_n_r, psd_n_i                                  # (F, C, C)
        xr, xi = psd_s_r, psd_s_i                                  # (F, C, C)

        def gj_step(k, carry):
            ar, ai, xr, xi = carry
            prr = ar[:, k, :]                                      # (F, C)
            pri = ai[:, k, :]
            pxr = xr[:, k, :]
            pxi = xi[:, k, :]
            d = prr[:, k] ** 2 + pri[:, k] ** 2                    # (F,)
            inv_r = prr[:, k] / d
            inv_i = -pri[:, k] / d
            srr = prr * inv_r[:, None] - pri * inv_i[:, None]
            sri = prr * inv_i[:, None] + pri * inv_r[:, None]
            sxr = pxr * inv_r[:, None] - pxi * inv_i[:, None]
            sxi = pxr * inv_i[:, None] + pxi * inv_r[:, None]
            fr = ar[:, :, k] * (1.0 - eye[k])[None, :]             # (F, C)
            fi = ai[:, :, k] * (1.0 - eye[k])[None, :]
            new_ar = ar - (fr[:, :, None] * srr[:, None, :] - fi[:, :, None] * sri[:, None, :])
            new_ai = ai - (fr[:, :, None] * sri[:, None, :] + fi[:, :, None] * srr[:, None, :])
            new_xr = xr - (fr[:, :, None] * sxr[:, None, :] - fi[:, :, None] * sxi[:, None, :])
            new_xi = xi - (fr[:, :, None] * sxi[:, None, :] + fi[:, :, None] * sxr[:, None, :])
            new_ar = new_ar.at[:, k, :].set(srr)
            new_ai = new_ai.at[:, k, :].set(sri)
            new_xr = new_xr.at[:, k, :].set(sxr)
            new_xi = new_xi.at[:, k, :].set(sxi)
            return new_ar, new_ai, new_xr, new_xi

        for k in range(C):
            ar, ai, xr, xi = gj_step(k, (ar, ai, xr, xi))
        num_r, num_i = xr, xi                                      # inv(N) @ S

        tr_r = jnp.trace(num_r, axis1=-2, axis2=-1)                # (F,)
        tr_i = jnp.trace(num_i, axis1=-2, axis2=-1)
        den = tr_r ** 2 + tr_i ** 2 + EPS
        itr_r = (tr_r + EPS) / den
        itr_i = -tr_i / den
        wsm_r = num_r * itr_r[:, None, None] - num_i * itr_i[:, None, None]
        wsm_i = num_r * itr_i[:, None, None] + num_i * itr_r[:, None, None]
        ws_r = wsm_r @ u                                           # (F, C)
        ws_i = wsm_i @ u

        # ---- beamform: enh[f,t] = sum_c conj(ws)[f,c] x[f,c,t] ----
        enh_r = (ws_r[:, :, None] * dr).sum(1) + (ws_i[:, :, None] * di).sum(1)
        enh_i = (ws_r[:, :, None] * di).sum(1) - (ws_i[:, :, None] * dr).sum(1)
        out = jnp.stack([enh_r.T, enh_i.T], axis=-1)               # (T, F, 2)
        return out[None]                                           # (1, T, F, 2)

    devices = jax.devices()[:8]
    mesh = Mesh(np.asarray(devices), ("b",))
    fn = jax.jit(shard_map(
        per_core, mesh=mesh,
        in_specs=(P("b"), P("b"), P("b"), P("b"), P(), P(), P(), P()),
        out_specs=P("b"),
        check_rep=False,
    ))
    return fn


def _norm_masks(mask):
    # (B,F,C,T) -> channel mean, normalized over T: (B,F,T)
    m = np.mean(mask, axis=-2, dtype=np.float32)
    return m / (m.sum(axis=-1, keepdims=True) + EPS)


def _kernel_host(data_real, data_imag, mask_speech, mask_noise,
                 mlp_w, mlp_b, gvec_w, gvec_b):
    """Numpy fallback (same math)."""
    data = np.transpose(data_real + 1j * data_imag, (0, 3, 2, 1)).astype(np.complex64)

    def psd(mask):
        m = _norm_masks(mask)
        return np.einsum('bfct,bft,bfet->bfce', data, m.astype(data.dtype),
                         np.conj(data))

    psd_s = psd(mask_speech)
    psd_n = psd(mask_noise)

    eye = np.eye(C, dtype=bool)
    z = np.where(eye, np.zeros((), psd_s.dtype), psd_s)
    p = np.swapaxes(z.sum(axis=-1) / (C - 1), -1, -2)
    feat = np.sqrt(p.real ** 2 + p.imag ** 2)
    mlp = np.tanh(feat @ mlp_w + mlp_b)
    e = (mlp @ gvec_w)[..., 0] + gvec_b[0]
    e = SCALING * e
    e = e - e.max(axis=-1, keepdims=True)
    ex = np.exp(e)
    u = ex / ex.sum(axis=-1, keepdims=True)

    num = np.linalg.inv(psd_n.astype(np.complex128)).astype(np.complex64) @ psd_s
    tr = np.einsum('bfcc->bf', num)
    wsm = num / (tr[..., None, None] + EPS)
    ws = np.einsum('bfec,bc->bfe', wsm, u.astype(wsm.dtype))
    enh = np.einsum('bfc,bfct->bft', np.conj(ws), data)
    enh = np.swapaxes(enh, -1, -2)
    return np.stack([enh.real, enh.imag], axis=-1).astype(np.float32)


def kernel(data_real, data_imag, mask_speech, mask_noise,
           mlp_w, mlp_b, gvec_w, gvec_b, ilens=None, **_unused):
    global _JAX_FN
    data_real = np.asarray(data_real, np.float32)
    data_imag = np.asarray(data_imag, np.float32)
    mask_speech = np.asarray(mask_speech, np.float32)
    mask_noise = np.asarray(mask_noise, np.float32)
    mlp_w = np.asarray(mlp_w, np.float32)
    mlp_b = np.asarray(mlp_b, np.float32)
    gvec_w = np.asarray(gvec_w, np.float32)
    gvec_b = np.asarray(gvec_b, np.float32)
    try:
        if _JAX_FN is None:
            _JAX_FN = _build_jax_fn()
        m_s = _norm_masks(mask_speech)
        m_n = _norm_masks(mask_noise)
        out = np.asarray(_JAX_FN(data_real, data_imag, m_s, m_n,
                                 mlp_w, mlp_b, gvec_w, gvec_b))
        return out.astype(np.float32)
    except Exception:
        return _kernel_host(data_real, data_imag, mask_speech, mask_noise,
                            mlp_w, mlp_b, gvec_w, gvec_b)


# revision 3
# speedup vs baseline: 1.7039x; 1.1263x over previous
"""DNN MVDR Beamformer — Trainium2, 8 NeuronCores.

Strategy: data-parallel over B (B=8 -> one batch element per core) via
jax shard_map on the 8 axon-attached NeuronCores; tiny MLP params
replicated. Per-core computation (PSD estimation, attention reference,
MVDR Gauss-Jordan solve, beamforming) is expressed in real-arithmetic
JAX (no complex dtypes, no LAPACK custom calls) so it lowers cleanly
through neuronx-cc. All contractions are written in the native (T,C,F)
layout of the input data so the kernel contains no large transposes;
F-batched contractions use broadcast-multiply + reduce (vector-engine
friendly) instead of 8x8-batched matmuls, a pathological PE shape.

Host-side prep (cheap, ~1% of FLOPs): the masks' channel mean +
T-normalization, shipped as (B,T,F) — this also halves host->device
transfer volume, which dominates wall time through the axon tunnel.
"""

import numpy as np

EPS = 1e-15
SCALING = 2.0
B, T, C, F, A = 8, 512, 8, 257, 320

_JAX_FN = None  # compiled shard_map callable, built lazily


def _build_jax_fn():
    import jax
    import jax.numpy as jnp
    from jax.sharding import Mesh, PartitionSpec as P
    from jax.experimental.shard_map import shard_map

    def per_core(data_real, data_imag, m_speech, m_noise,
                 mlp_w, mlp_b, gvec_w, gvec_b):
        # per-core shapes: data (1,T,C,F), masks (1,T,F) pre-normalized
        dr = data_real[0]                                          # (T, C, F)
        di = data_imag[0]
        ms = m_speech[0]                                           # (T, F)
        mn = m_noise[0]

        def psd(m):
            wr = dr * m[:, None, :]                                # (T, C, F)
            wi = di * m[:, None, :]
            # S[c,e,f] = sum_t w[t,c,f] * x[t,e,f]
            sr = (wr[:, :, None, :] * dr[:, None, :, :]).sum(0) \
               + (wi[:, :, None, :] * di[:, None, :, :]).sum(0)
            si = (wi[:, :, None, :] * dr[:, None, :, :]).sum(0) \
               - (wr[:, :, None, :] * di[:, None, :, :]).sum(0)
            return sr, si                                          # (C, C, F)

        psd_s_r, psd_s_i = psd(ms)
        psd_n_r, psd_n_i = psd(mn)

        # ---- attention reference -> u (C,) ----
        eye = jnp.eye(C, dtype=jnp.float32)                        # (c, e)
        zdiag = (1.0 - eye)[:, :, None]                            # (C, C, 1)
        pr = (psd_s_r * zdiag).sum(1) / (C - 1)                    # (C, F)
        pi = (psd_s_i * zdiag).sum(1) / (C - 1)
        feat = jnp.sqrt(pr * pr + pi * pi)                         # (C, F)
        mlp = jnp.tanh(feat @ mlp_w + mlp_b)                       # (C, A)
        e = (mlp @ gvec_w)[:, 0] + gvec_b[0]                       # (C,)
        e = SCALING * e
        e = e - e.max()
        ex = jnp.exp(e)
        u = ex / ex.sum()                                          # (C,)

        # ---- MVDR: solve psd_n @ X = psd_s per f (8x8 complex GJ) ----
        # layout (row c, col e, f)
        ar, ai = psd_n_r, psd_n_i                                  # (C, C, F)
        xr, xi = psd_s_r, psd_s_i

        for k in range(C):
            prr = ar[k, :, :]                                      # (C, F) pivot row
            pri = ai[k, :, :]
            pxr = xr[k, :, :]
            pxi = xi[k, :, :]
            d = prr[k] ** 2 + pri[k] ** 2                          # (F,)
            inv_r = prr[k] / d
            inv_i = -pri[k] / d
            srr = prr * inv_r[None, :] - pri * inv_i[None, :]      # (C, F)
            sri = prr * inv_i[None, :] + pri * inv_r[None, :]
            sxr = pxr * inv_r[None, :] - pxi * inv_i[None, :]
            sxi = pxr * inv_i[None, :] + pxi * inv_r[None, :]
            fr = ar[:, k, :] * (1.0 - eye[k])[:, None]             # (C, F) col factors
            fi = ai[:, k, :] * (1.0 - eye[k])[:, None]
            ar = ar - (fr[:, None, :] * srr[None, :, :] - fi[:, None, :] * sri[None, :, :])
            ai = ai - (fr[:, None, :] * sri[None, :, :] + fi[:, None, :] * srr[None, :, :])
            xr = xr - (fr[:, None, :] * sxr[None, :, :] - fi[:, None, :] * sxi[None, :, :])
            xi = xi - (fr[:, None, :] * sxi[None, :, :] + fi[:, None, :] * sxr[None, :, :])
            ar = ar.at[k, :, :].set(srr)
            ai = ai.at[k, :, :].set(sri)
            xr = xr.at[k, :, :].set(sxr)
            xi = xi.at[k, :, :].set(sxi)
        num_r, num_i = xr, xi                                      # inv(N) @ S, (C,C,F)

        tr_r = jnp.einsum('ccf->f', num_r)                         # (F,)
        tr_i = jnp.einsum('ccf->f', num_i)
        den = tr_r ** 2 + tr_i ** 2 + EPS
        itr_r = (tr_r + EPS) / den
        itr_i = -tr_i / den
        wsm_r = num_r * itr_r[None, None, :] - num_i * itr_i[None, None, :]
        wsm_i = num_r * itr_i[None, None, :] + num_i * itr_r[None, None, :]
        # ws[e,f] = sum_c wsm[e,c,f] * u[c]   (u real; wsm rows=e after solve)
        ws_r = (wsm_r * u[None, :, None]).sum(1)                   # (C, F) -> (e, f)
        ws_i = (wsm_i * u[None, :, None]).sum(1)

        # ---- beamform: enh[t,f] = sum_c conj(ws)[c,f] x[t,c,f] ----
        enh_r = (ws_r[None, :, :] * dr).sum(1) + (ws_i[None, :, :] * di).sum(1)
        enh_i = (ws_r[None, :, :] * di).sum(1) - (ws_i[None, :, :] * dr).sum(1)
        out = jnp.stack([enh_r, enh_i], axis=-1)                   # (T, F, 2)
        return out[None]                                           # (1, T, F, 2)

    devices = jax.devices()[:8]
    mesh = Mesh(np.asarray(devices), ("b",))
    fn = jax.jit(shard_map(
        per_core, mesh=mesh,
        in_specs=(P("b"), P("b"), P("b"), P("b"), P(), P(), P(), P()),
        out_specs=P("b"),
        check_rep=False,
    ))
    return fn


def _norm_masks(mask):
    # (B,F,C,T) -> channel mean, normalized over T, transposed: (B,T,F)
    m = np.mean(mask, axis=-2, dtype=np.float32)                   # (B,F,T)
    m = m / (m.sum(axis=-1, keepdims=True) + EPS)
    return np.ascontiguousarray(np.swapaxes(m, 1, 2))              # (B,T,F)


def _kernel_host(data_real, data_imag, mask_speech, mask_noise,
                 mlp_w, mlp_b, gvec_w, gvec_b):
    """Numpy fallback (same math)."""
    data = np.transpose(data_real + 1j * data_imag, (0, 3, 2, 1)).astype(np.complex64)

    def psd(mask):
        m = np.mean(mask, axis=-2, dtype=np.float32)
        m = m / (m.sum(axis=-1, keepdims=True) + EPS)
        return np.einsum('bfct,bft,bfet->bfce', data, m.astype(data.dtype),
                         np.conj(data))

    psd_s = psd(mask_speech)
    psd_n = psd(mask_noise)

    eye = np.eye(C, dtype=bool)
    z = np.where(eye, np.zeros((), psd_s.dtype), psd_s)
    p = np.swapaxes(z.sum(axis=-1) / (C - 1), -1, -2)
    feat = np.sqrt(p.real ** 2 + p.imag ** 2)
    mlp = np.tanh(feat @ mlp_w + mlp_b)
    e = (mlp @ gvec_w)[..., 0] + gvec_b[0]
    e = SCALING * e
    e = e - e.max(axis=-1, keepdims=True)
    ex = np.exp(e)
    u = ex / ex.sum(axis=-1, keepdims=True)

    num = np.linalg.inv(psd_n.astype(np.complex128)).astype(np.complex64) @ psd_s
    tr = np.einsum('bfcc->bf', num)
    wsm = num / (tr[..., None, None] + EPS)
    ws = np.einsum('bfec,bc->bfe', wsm, u.astype(wsm.dtype))
    enh = np.einsum('bfc,bfct->bft', np.conj(ws), data)
    enh = np.swapaxes(enh, -1, -2)
    return np.stack([enh.real, enh.imag], axis=-1).astype(np.float32)


def kernel(data_real, data_imag, mask_speech, mask_noise,
           mlp_w, mlp_b, gvec_w, gvec_b, ilens=None, **_unused):
    global _JAX_FN
    data_real = np.asarray(data_real, np.float32)
    data_imag = np.asarray(data_imag, np.float32)
    mask_speech = np.asarray(mask_speech, np.float32)
    mask_noise = np.asarray(mask_noise, np.float32)
    mlp_w = np.asarray(mlp_w, np.float32)
    mlp_b = np.asarray(mlp_b, np.float32)
    gvec_w = np.asarray(gvec_w, np.float32)
    gvec_b = np.asarray(gvec_b, np.float32)
    try:
        if _JAX_FN is None:
            _JAX_FN = _build_jax_fn()
        m_s = _norm_masks(mask_speech)
        m_n = _norm_masks(mask_noise)
        out = np.asarray(_JAX_FN(data_real, data_imag, m_s, m_n,
                                 mlp_w, mlp_b, gvec_w, gvec_b))
        return out.astype(np.float32)
    except Exception:
        return _kernel_host(data_real, data_imag, mask_speech, mask_noise,
                            mlp_w, mlp_b, gvec_w, gvec_b)
